# revision 21
# baseline (speedup 1.0000x reference)
"""Trainium2 Bass kernel for a single AttnDecoderRNN step (B=8, H=1024, L=512, V=50000).

Strategy (8 NeuronCores, tensor-parallel):
  - Dominant cost is streaming out_w (V x H). Vocab dim is sharded 8 ways
    and host-converted to bf16 [H, V/8] -> 12.8 MB/core, prefetched into
    SBUF in 512 KB DMAs on the HWDGE (sync) queue from t=0.
  - The small "front" (attention + GRU) is sharded too: attention over L
    (64 rows/core, bf16), combine + GRU over H (128 cols/core, f32 for
    accuracy). Cross-core stitching uses AllGather + local reduction
    (AllGather is ~5us on-chip vs ~23us for AllReduce): attention
    numerator/denominator, g^T, h_new^T, softmax denominator. A dummy
    0-dependency collective issued first absorbs the ~30us ncfw init.
  - Front inputs are host-packed into a handful of large DMAs on the
    gpsimd (SWDGE) queue so nothing queues behind the stream.
  - log_softmax: exp per vocab tile with accum_out partial sums, one
    AllGather of the 8 partial denominators, logp = logits - log(den).
    Numerics: logits are bounded (|logit| <~ 40) so no max subtraction.
Outputs per core: logp shard [8, 6250] f32 and h_new shard [8, 128] f32.
"""

import os
import sys

import numpy as np

sys.path.insert(0, "/opt/trn_rl_repo")

import concourse.bass as bass  # noqa: E402,F401
import concourse.mybir as mybir  # noqa: E402
from concourse import bacc, tile  # noqa: E402
from concourse.bass_utils import run_bass_kernel_spmd  # noqa: E402

F32 = mybir.dt.float32
BF16 = mybir.dt.bfloat16
AF = mybir.ActivationFunctionType

V, H, L, B = 50000, 1024, 512, 8
NCORES = 8
VC = V // NCORES  # 6250 vocab rows per core
LC = L // NCORES  # 64 attention positions per core
HC = H // NCORES  # 128 hidden dims per core
KC = H // 128  # 8 contraction chunks of 128
NT = 512  # vocab tile width (one PSUM bank)
NVT = (VC + NT - 1) // NT  # 13 tiles (12 full + 106)
LAST_W = VC - (NVT - 1) * NT  # 106
GROUPS = [(0, 4), (4, 4), (8, 4), (12, 1)]  # (start vt, count)
RG = [list(range(NCORES))]
STREAM_BUFS = int(os.environ.get("KERNEL_STREAM_BUFS", "20"))
G3 = 3 * HC

# column offsets inside the packed [128, 8320] f32 front blob
PF_XT, PF_HT, PF_CW, PF_WIH, PF_WHH, PF_END = 0, 64, 128, 2176, 5248, 8320
# column offsets inside the packed [8, 1096] f32 bias blob
PB_AB, PB_CB, PB_BIH, PB_BHH, PB_HSL, PB_EYE, PB_END = 0, 64, 192, 576, 960, 1088, 1096

_CACHE = {}


def _build():
    nc = bacc.Bacc(None, target_bir_lowering=False, num_devices=NCORES)

    dp = nc.declare_dram_parameter
    d_pf = dp("pack_f32", [128, PF_END], F32, isOutput=False)
    d_pb = dp("bias_pack", [B, PB_END], F32, isOutput=False)
    d_aw = dp("attn_wT", [128, 16 * LC], BF16, isOutput=False)
    d_enc = dp("enc", [LC, B * H], BF16, isOutput=False)
    d_sm = dp("small_bf", [B, 16], BF16, isOutput=False)
    d_owT = dp("out_wT", [H, VC], BF16, isOutput=False)
    d_ob = dp("out_b", [1, VC], BF16, isOutput=False)

    d_logp = dp("logp_out", [B, VC], F32, isOutput=True)
    d_hnew = dp("hnew_out", [B, HC], F32, isOutput=True)

    with tile.TileContext(nc) as tc:
        with (
            tc.tile_pool(name="const", bufs=1) as cpool,
            tc.tile_pool(name="work", bufs=1) as wpool,
            tc.tile_pool(name="stream", bufs=STREAM_BUFS) as spool,
            tc.tile_pool(name="psum", bufs=5, space="PSUM") as ppool,
            tc.tile_pool(name="papp", bufs=1, space="PSUM") as apool,
            tc.tile_pool(name="psmall", bufs=2, space="PSUM") as pspool,
            tc.tile_pool(name="dram", bufs=1, space="DRAM") as dpool,
        ):
            sdma = nc.sync.dma_start  # big stream (HWDGE)
            fdma = nc.gpsimd.dma_start  # front/small transfers (SWDGE)

            # collective bounce buffers (DRAM, internal). Per-core block in
            # cc1 padded to 1152 = 9*128 rows so the AllGather output
            # factors as (c q p) for a single strided reload.
            cc1_in = dpool.tile([1152, B], F32)
            cc1_out = dpool.tile([NCORES * 1152, B], F32)
            cc2_in = dpool.tile([128, B], F32)
            cc2_out = dpool.tile([H, B], F32)
            cc3_in = dpool.tile([128, B], F32)
            cc3_out = dpool.tile([H, B], F32)
            cc4_in = dpool.tile([B, 1], F32)
            cc4_out = dpool.tile([NCORES * B, 1], F32)

            # zero cc1's pad rows (sim NaN checker; AG just copies bytes)
            zz_t = wpool.tile([128, B], F32)
            nc.vector.memset(zz_t[:, :], 0.0)
            fdma(cc1_in[1025:1152, :], zz_t[0:127, :])

            # ---- packed front loads ----
            pf_t = cpool.tile([128, PF_END], F32)
            fdma(pf_t[:, 0:PF_CW], d_pf[:, 0:PF_CW])  # x^T|h^T first (attn)
            pb_t = cpool.tile([B, PB_END], F32)
            fdma(pb_t[:, :], d_pb[:, :])
            aw_t = cpool.tile([128, 16 * LC], BF16)
            fdma(aw_t[:, :], d_aw[:, :])
            enc_t = cpool.tile([LC, B * H], BF16)
            fdma(enc_t[:, :], d_enc[:, :])
            sm_t = cpool.tile([B, 16], BF16)
            fdma(sm_t[:, :], d_sm[:, :])
            fdma(pf_t[:, PF_CW:PF_END], d_pf[:, PF_CW:PF_END])  # comb/GRU w
            outb_t = cpool.tile([1, VC], BF16)
            sdma(outb_t[:, :], d_ob[:, :])

            xT_t = pf_t[:, PF_XT:PF_XT + KC * B]
            hT_t = pf_t[:, PF_HT:PF_HT + KC * B]
            cw_t = pf_t[:, PF_CW:PF_CW + 16 * HC]
            wih_t = pf_t[:, PF_WIH:PF_WIH + KC * G3]
            whh_t = pf_t[:, PF_WHH:PF_WHH + KC * G3]
            ab_t = pb_t[:, PB_AB:PB_AB + LC]
            cb_t = pb_t[:, PB_CB:PB_CB + HC]
            bih_t = pb_t[:, PB_BIH:PB_BIH + G3]
            bhh_t = pb_t[:, PB_BHH:PB_BHH + G3]
            hsl_t = pb_t[:, PB_HSL:PB_HSL + HC]
            eyef_t = pb_t[:, PB_EYE:PB_EYE + B]
            eye_t = sm_t[:, 0:B]
            ones_t = sm_t[0:1, B:2 * B]

            # bf16 copies of x^T / h^T for the (bf16) attention matmuls
            xh_bf = cpool.tile([128, 2 * KC * B], BF16)
            nc.vector.tensor_copy(xh_bf[:, :], pf_t[:, 0:2 * KC * B])
            xT_bf = xh_bf[:, 0:KC * B]
            hT_bf = xh_bf[:, KC * B:2 * KC * B]

            # ================= attention =================
            with nc.named_scope("attn"):
                ps_attn = pspool.tile([B, LC], F32, tag="ps_small")
                for k in range(16):
                    lhsT = xT_bf if k < KC else hT_bf
                    kk = k % KC
                    nc.tensor.matmul(
                        ps_attn[:, :],
                        lhsT[:, kk * B:(kk + 1) * B],
                        aw_t[:, k * LC:(k + 1) * LC],
                        start=(k == 0),
                        stop=(k == 15),
                    )
                al_t = wpool.tile([B, LC], F32)
                nc.vector.tensor_add(al_t[:, :], ps_attn[:, :], ab_t[:, :])
                # e = exp(scores), den = sum_l e (no max: |scores| < ~40)
                e_t = wpool.tile([B, LC], BF16)
                den_t = wpool.tile([B, 1], F32)
                nc.scalar.activation(
                    e_t[:, :], al_t[:, :], AF.Exp, accum_out=den_t[:, :]
                )
                ps_eT = pspool.tile([LC, B], BF16, tag="ps_small")
                nc.tensor.transpose(ps_eT[:, :], e_t[:, :], eye_t[0:B, 0:B])
                eT_t = wpool.tile([LC, B], BF16)
                nc.vector.tensor_copy(eT_t[:, :], ps_eT[:, :])

                # partial applied^T: appT[h,b] = sum_l enc[b,l,h]*e[b,l]
                ps_appT = apool.tile([128, KC * B], F32)
                for b in range(B):
                    for ch in range(KC):
                        nc.tensor.matmul(
                            ps_appT[:, ch * B + b:ch * B + b + 1],
                            enc_t[:, b * H + ch * 128: b * H + (ch + 1) * 128],
                            eT_t[:, b:b + 1],
                            start=True,
                            stop=True,
                        )
                appT_t = wpool.tile([128, KC * B], F32)
                nc.vector.tensor_copy(appT_t[:, :], ps_appT[:, :])
                # AllGather partials, reduce locally (AG is much cheaper
                # than AllReduce on-chip)
                fdma(
                    cc1_in[0:H, :].rearrange("(k p) b -> p k b", p=128),
                    appT_t[:, :].rearrange("p (k b) -> p k b", k=KC),
                )
                fdma(cc1_in[H:H + 1, :], den_t[:, :])
                nc.gpsimd.collective_compute(
                    "AllGather", mybir.AluOpType.bypass, replica_groups=RG,
                    ins=[cc1_in.opt()], outs=[cc1_out.opt()],
                )
                agbuf = wpool.tile([128, NCORES * 72], F32)
                fdma(
                    agbuf[:, :].rearrange("p (cq b) -> p cq b", b=B),
                    cc1_out[:, :].rearrange("(cq p) b -> p cq b", p=128),
                )
                appT_g = wpool.tile([128, KC * B], F32)
                nc.vector.tensor_add(
                    appT_g[:, :], agbuf[:, 0:KC * B],
                    agbuf[:, 72:72 + KC * B],
                )
                for c in range(2, NCORES):
                    nc.vector.tensor_add(
                        appT_g[:, :], appT_g[:, :],
                        agbuf[:, c * 72:c * 72 + KC * B],
                    )
                den8_t = wpool.tile([B, NCORES], F32)
                fdma(
                    den8_t[:, :],
                    cc1_out.rearrange("(c r) b -> c r b", r=1152)[:, H, :]
                    .rearrange("c b -> b c"),
                )
                denl_t = wpool.tile([B, 1], F32)
                nc.vector.reduce_sum(
                    denl_t[:, :], den8_t[:, :], axis=mybir.AxisListType.X
                )
                recip_t = wpool.tile([B, 1], F32)
                nc.vector.reciprocal(recip_t[:, :], denl_t[:, :])

                # xa_t = [x^T | UNSCALED applied^T]; 1/den applied after the
                # combine matmul (linear in applied) as a per-partition scalar
                xa_t = cpool.tile([128, 2 * KC * B], F32)
                nc.vector.tensor_copy(xa_t[:, 0:KC * B], xT_t[:, :])
                nc.vector.tensor_copy(xa_t[:, KC * B:2 * KC * B], appT_g[:, :])

            # ================= combine + relu =================
            with nc.named_scope("comb"):
                ps_gx = pspool.tile([B, HC], F32, tag="ps_small")
                ps_ga = pspool.tile([B, HC], F32, tag="ps_small")
                for k in range(KC):
                    nc.tensor.matmul(
                        ps_gx[:, :],
                        xa_t[:, k * B:(k + 1) * B],
                        cw_t[:, k * HC:(k + 1) * HC],
                        start=(k == 0),
                        stop=(k == KC - 1),
                    )
                for k in range(KC, 16):
                    nc.tensor.matmul(
                        ps_ga[:, :],
                        xa_t[:, k * B:(k + 1) * B],
                        cw_t[:, k * HC:(k + 1) * HC],
                        start=(k == KC),
                        stop=(k == 15),
                    )
                gsc_t = wpool.tile([B, HC], F32)
                nc.vector.tensor_scalar_mul(
                    gsc_t[:, :], ps_ga[:, :], recip_t[:, :]
                )
                gpre_t = wpool.tile([B, HC], F32)
                nc.vector.tensor_add(gpre_t[:, :], gsc_t[:, :], ps_gx[:, :])
                gpre2_t = wpool.tile([B, HC], F32)
                nc.vector.tensor_add(gpre2_t[:, :], gpre_t[:, :], cb_t[:, :])
                g_t = wpool.tile([B, HC], F32)
                nc.scalar.activation(g_t[:, :], gpre2_t[:, :], AF.Relu)

                ps_gT = pspool.tile([HC, B], F32, tag="ps_small")
                nc.tensor.transpose(ps_gT[:, :], g_t[:, :], eyef_t[0:B, 0:B])
                gT_sb = wpool.tile([HC, B], F32)
                nc.vector.tensor_copy(gT_sb[:, :], ps_gT[:, :])
                fdma(cc2_in[:, :], gT_sb[:, :])
                nc.gpsimd.collective_compute(
                    "AllGather", mybir.AluOpType.bypass, replica_groups=RG,
                    ins=[cc2_in.opt()], outs=[cc2_out.opt()],
                )
                gT_t = cpool.tile([128, KC * B], F32)
                fdma(
                    gT_t[:, :].rearrange("p (k b) -> p k b", k=KC),
                    cc2_out[:, :].rearrange("(k p) b -> p k b", p=128),
                )

            # ================= GRU =================
            with nc.named_scope("gru"):
                ps_gi = pspool.tile([B, G3], F32, tag="ps_small")
                ps_gh = pspool.tile([B, G3], F32, tag="ps_small")
                for k in range(KC):
                    nc.tensor.matmul(
                        ps_gi[:, :], gT_t[:, k * B:(k + 1) * B],
                        wih_t[:, k * G3:(k + 1) * G3],
                        start=(k == 0), stop=(k == KC - 1),
                    )
                for k in range(KC):
                    nc.tensor.matmul(
                        ps_gh[:, :], hT_t[:, k * B:(k + 1) * B],
                        whh_t[:, k * G3:(k + 1) * G3],
                        start=(k == 0), stop=(k == KC - 1),
                    )
                gi_t = wpool.tile([B, G3], F32)
                nc.vector.tensor_add(gi_t[:, :], ps_gi[:, :], bih_t[:, :])
                gh_t = wpool.tile([B, G3], F32)
                nc.vector.tensor_add(gh_t[:, :], ps_gh[:, :], bhh_t[:, :])

                # r and z gates share one add + one sigmoid over [8, 256]
                rz_pre = wpool.tile([B, 2 * HC], F32)
                nc.vector.tensor_add(
                    rz_pre[:, :], gi_t[:, 0:2 * HC], gh_t[:, 0:2 * HC]
                )
                rz_t = wpool.tile([B, 2 * HC], F32)
                nc.scalar.activation(rz_t[:, :], rz_pre[:, :], AF.Sigmoid)
                rhn_t = wpool.tile([B, HC], F32)
                nc.vector.tensor_mul(
                    rhn_t[:, :], rz_t[:, 0:HC], gh_t[:, 2 * HC:G3]
                )
                npre_t = wpool.tile([B, HC], F32)
                nc.vector.tensor_add(
                    npre_t[:, :], gi_t[:, 2 * HC:G3], rhn_t[:, :]
                )
                n_t = wpool.tile([B, HC], F32)
                nc.scalar.activation(n_t[:, :], npre_t[:, :], AF.Tanh)
                # h_new = n + z * (h - n)
                d_t = wpool.tile([B, HC], F32)
                nc.vector.tensor_sub(d_t[:, :], hsl_t[:, :], n_t[:, :])
                zd_t = wpool.tile([B, HC], F32)
                nc.vector.tensor_mul(zd_t[:, :], rz_t[:, HC:2 * HC], d_t[:, :])
                hnew_t = wpool.tile([B, HC], F32)
                nc.vector.tensor_add(hnew_t[:, :], n_t[:, :], zd_t[:, :])
                fdma(d_hnew[:, :], hnew_t[:, :])

                # h_new^T shard -> AllGather (f32) -> bf16 chunks for stream
                ps_hT = pspool.tile([HC, B], F32, tag="ps_small")
                nc.tensor.transpose(ps_hT[:, :], hnew_t[:, :], eyef_t[0:B, 0:B])
                hT_sb = wpool.tile([HC, B], F32)
                nc.vector.tensor_copy(hT_sb[:, :], ps_hT[:, :])
                fdma(cc3_in[:, :], hT_sb[:, :])
                nc.gpsimd.collective_compute(
                    "AllGather", mybir.AluOpType.bypass, replica_groups=RG,
                    ins=[cc3_in.opt()], outs=[cc3_out.opt()],
                )
                hn32_t = cpool.tile([128, KC * B], F32)
                fdma(
                    hn32_t[:, :].rearrange("p (k b) -> p k b", k=KC),
                    cc3_out[:, :].rearrange("(k p) b -> p k b", p=128),
                )
                hnT_t = cpool.tile([128, KC * B], BF16)
                nc.vector.tensor_copy(hnT_t[:, :], hn32_t[:, :])

            # ================= vocab stream =================
            with nc.named_scope("warm"):
                ps_w = pspool.tile([B, NT], F32, tag="ps_small")
                for i in range(12):
                    nc.tensor.matmul(
                        ps_w[:, :], hT_sb[:, :], pf_t[:, PF_CW + i * NT:
                                                      PF_CW + (i + 1) * NT],
                        start=(i == 0), stop=(i == 11),
                    )
                wsink_t = wpool.tile([B, NT], F32)
                nc.vector.tensor_copy(wsink_t[:, :], ps_w[:, :])
            with nc.named_scope("stream"):
                logits_t = cpool.tile([B, VC], F32)
                stats_t = cpool.tile([B, NVT], F32)
                for (s, cnt) in GROUPS:
                    gw = min(VC, (s + cnt) * NT) - s * NT
                    ps = [
                        ppool.tile([B, NT], F32, tag="ps_l", bufs=5,
                                   name=f"ps{s}_{j}")
                        for j in range(cnt)
                    ]
                    for k in range(KC):
                        rhs_t = spool.tile([128, 4 * NT], BF16, tag="rhs")
                        sdma(
                            rhs_t[:, 0:gw],
                            d_owT[k * 128:(k + 1) * 128, s * NT:s * NT + gw],
                        )
                        for j in range(cnt):
                            w = min(NT, gw - j * NT)
                            nc.tensor.matmul(
                                ps[j][:, 0:w],
                                hnT_t[:, k * B:(k + 1) * B],
                                rhs_t[:, j * NT:j * NT + w],
                                start=(k == 0),
                                stop=False,
                            )
                    for j in range(cnt):
                        vt = s + j
                        w = NT if vt < NVT - 1 else LAST_W
                        # + out_b via a K=1 ones row; closes the psum group
                        nc.tensor.matmul(
                            ps[j][:, 0:w], ones_t[:, :],
                            outb_t[:, vt * NT:vt * NT + w],
                            start=False, stop=True,
                        )
                        nc.scalar.activation(
                            logits_t[:, vt * NT:vt * NT + w], ps[j][:, 0:w],
                            AF.Copy,
                        )
                        esc_t = wpool.tile([B, NT], F32, tag="esc", bufs=3)
                        nc.scalar.activation(
                            esc_t[:, 0:w], ps[j][:, 0:w], AF.Exp,
                            accum_out=stats_t[:, vt:vt + 1],
                        )

            # ================= softmax tail =================
            with nc.named_scope("tail"):
                denv_t = wpool.tile([B, 1], F32)
                nc.vector.reduce_sum(
                    denv_t[:, :], stats_t[:, :], axis=mybir.AxisListType.X
                )
                fdma(cc4_in[:, :], denv_t[:, :])
                nc.gpsimd.collective_compute(
                    "AllGather", mybir.AluOpType.bypass, replica_groups=RG,
                    ins=[cc4_in.opt()], outs=[cc4_out.opt()],
                )
                dall_t = wpool.tile([B, NCORES], F32)
                fdma(
                    dall_t[:, :],
                    cc4_out[:, :].rearrange("(c b) one -> b c one", b=B),
                )
                deng_t = wpool.tile([B, 1], F32)
                nc.vector.reduce_sum(
                    deng_t[:, :], dall_t[:, :], axis=mybir.AxisListType.X
                )
                logz_t = wpool.tile([B, 1], F32)
                nc.scalar.activation(logz_t[:, :], deng_t[:, :], AF.Ln)
                nc.vector.tensor_scalar_sub(
                    logits_t[:, :], logits_t[:, :], logz_t[:, :]
                )
                sdma(d_logp[:, :], logits_t[:, :])

    nc.compile()
    return nc


def _prep_inputs(input, hidden, encoder_outputs, emb, attn_w, attn_b,
                 comb_w, comb_b, w_ih, w_hh, b_ih, b_hh, out_w, out_b):
    import ml_dtypes

    f32 = np.float32
    bf16 = ml_dtypes.bfloat16
    c_ = np.ascontiguousarray

    def chunked(a, width):
        # [K*128, width] -> [128, K*width], chunk k at cols [k*w,(k+1)*w)
        k = a.shape[0] // 128
        return a.reshape(k, 128, -1).transpose(1, 0, 2).reshape(128, -1)

    ids = np.asarray(input).reshape(-1).astype(np.int64)
    x = np.asarray(emb)[ids].astype(f32)  # [B, H] embedding row gather
    h = np.asarray(hidden, dtype=f32)[0]  # [B, H]
    enc = np.asarray(encoder_outputs, dtype=f32)
    awT = np.asarray(attn_w, dtype=f32).T  # [2H, L]
    cwT = np.asarray(comb_w, dtype=f32).T  # [2H, H]
    wihT = np.asarray(w_ih, dtype=f32).T  # [H, 3H]
    whhT = np.asarray(w_hh, dtype=f32).T
    ab = np.asarray(attn_b, dtype=f32)
    cb = np.asarray(comb_b, dtype=f32)
    bih = np.asarray(b_ih, dtype=f32)
    bhh = np.asarray(b_hh, dtype=f32)
    ow = np.asarray(out_w, dtype=f32)
    ob = np.asarray(out_b, dtype=f32)

    xTc = chunked(c_(x.T), B)  # [128, 64]
    hTc = chunked(c_(h.T), B)

    small = np.zeros((B, 16), dtype=bf16)
    small[:, 0:B] = np.eye(B, dtype=bf16)
    small[0, B:16] = np.ones(B, dtype=bf16)

    def rep(v):
        return np.broadcast_to(v[None, :], (B, v.shape[0])).astype(f32)

    in_maps = []
    for c in range(NCORES):
        lsl = slice(c * LC, (c + 1) * LC)
        hsl = slice(c * HC, (c + 1) * HC)
        vsl = slice(c * VC, (c + 1) * VC)
        gcols = np.concatenate(
            [np.arange(g * H + c * HC, g * H + (c + 1) * HC) for g in range(3)]
        )
        pf = np.empty((128, PF_END), dtype=f32)
        pf[:, PF_XT:PF_HT] = xTc
        pf[:, PF_HT:PF_CW] = hTc
        pf[:, PF_CW:PF_WIH] = chunked(c_(cwT[:, hsl]), HC)
        pf[:, PF_WIH:PF_WHH] = chunked(c_(wihT[:, gcols]), G3)
        pf[:, PF_WHH:PF_END] = chunked(c_(whhT[:, gcols]), G3)

        pb = np.empty((B, PB_END), dtype=f32)
        pb[:, PB_AB:PB_CB] = rep(ab[lsl])
        pb[:, PB_CB:PB_BIH] = rep(cb[hsl])
        pb[:, PB_BIH:PB_BHH] = rep(bih[gcols])
        pb[:, PB_BHH:PB_HSL] = rep(bhh[gcols])
        pb[:, PB_HSL:PB_EYE] = h[:, hsl]
        pb[:, PB_EYE:PB_END] = np.eye(B, dtype=f32)

        in_maps.append({
            "pack_f32": pf,
            "bias_pack": pb,
            "attn_wT": c_(chunked(c_(awT[:, lsl]), LC).astype(bf16)),
            "enc": c_(enc[:, lsl, :].transpose(1, 0, 2)
                      .reshape(LC, B * H).astype(bf16)),
            "small_bf": small,
            "out_wT": c_(ow[vsl, :].T.astype(bf16)),
            "out_b": c_(ob[vsl][None, :].astype(bf16)),
        })
    return in_maps


def _enable_axon_ntff_hook():
    """The agent image's antenv lacks axon_hooks; shim it and install the
    ctypes NTFF hook so run_bass_kernel_spmd(trace=True) works under axon."""
    import types

    try:
        import antenv.axon_hooks  # noqa: F401
        return
    except ImportError:
        pass
    import antenv

    mod = types.ModuleType("antenv.axon_hooks")
    _h = [None]
    mod.get_axon_ntff_profile_hook = lambda: _h[0]
    mod.set_axon_ntff_profile_hook = lambda hook: _h.__setitem__(0, hook)
    sys.modules["antenv.axon_hooks"] = mod
    antenv.axon_hooks = mod
    try:
        from trn_agent_boot.trn_boot import _ntff_profile_via_ctypes

        hook = _ntff_profile_via_ctypes("/opt/axon/libaxon_pjrt.so")
        if hook is not None:
            mod.set_axon_ntff_profile_hook(hook)
    except Exception as e:  # profiling-only convenience; never fatal
        print("ntff hook install failed:", e)


def kernel(**inputs):
    if "nc" not in _CACHE:
        _CACHE["nc"] = _build()
    nc = _CACHE["nc"]
    in_maps = _prep_inputs(**inputs)

    trace = bool(os.environ.get("KERNEL_PROFILE"))
    if trace:
        _enable_axon_ntff_hook()
        import concourse.bass_utils as _bu

        _bu.upload_artifacts = lambda tmpdir: "local://" + tmpdir
    res = run_bass_kernel_spmd(
        nc,
        in_maps,
        core_ids=list(range(NCORES)),
        trace=trace,
        trace_cores=list(range(NCORES)) if trace else None,
    )
    _CACHE["last_result"] = res

    logp = np.concatenate(
        [res.results[c]["logp_out"] for c in range(NCORES)], axis=1
    )
    hnew = np.concatenate(
        [res.results[c]["hnew_out"] for c in range(NCORES)], axis=1
    )
    return logp, hnew[None, :, :]


# revision 22
# speedup vs baseline: 1.4676x; 1.4676x over previous
"""Trainium2 Bass kernel for a single AttnDecoderRNN step (B=8, H=1024, L=512, V=50000).

Strategy (8 NeuronCores, tensor-parallel):
  - Dominant cost is streaming out_w (V x H). Vocab dim is sharded 8 ways
    and host-converted to bf16 [H, V/8] -> 12.8 MB/core, prefetched into
    SBUF in 512 KB DMAs on the HWDGE (sync) queue from t=0.
  - The small "front" (attention + GRU) is sharded too: attention over L
    (64 rows/core, bf16), combine + GRU over H (128 cols/core, f32 for
    accuracy). Cross-core stitching uses AllGather + local reduction
    (AllGather is ~5us on-chip vs ~23us for AllReduce): attention
    numerator/denominator, g^T, h_new^T, softmax denominator. A dummy
    0-dependency collective issued first absorbs the ~30us ncfw init.
  - Front inputs are host-packed into a handful of large DMAs on the
    gpsimd (SWDGE) queue so nothing queues behind the stream.
  - log_softmax: exp per vocab tile with accum_out partial sums, one
    AllGather of the 8 partial denominators, logp = logits - log(den).
    Numerics: logits are bounded (|logit| <~ 40) so no max subtraction.
Outputs per core: logp shard [8, 6250] f32 and h_new shard [8, 128] f32.
"""

import os
import sys

import numpy as np

sys.path.insert(0, "/opt/trn_rl_repo")

import concourse.bass as bass  # noqa: E402,F401
import concourse.mybir as mybir  # noqa: E402
from concourse import bacc, tile  # noqa: E402
from concourse.bass_utils import run_bass_kernel_spmd  # noqa: E402

F32 = mybir.dt.float32
BF16 = mybir.dt.bfloat16
AF = mybir.ActivationFunctionType

V, H, L, B = 50000, 1024, 512, 8
NCORES = 8
VC = V // NCORES  # 6250 vocab rows per core
LC = L // NCORES  # 64 attention positions per core
HC = H // NCORES  # 128 hidden dims per core
KC = H // 128  # 8 contraction chunks of 128
NT = 512  # vocab tile width (one PSUM bank)
NVT = (VC + NT - 1) // NT  # 13 tiles (12 full + 106)
LAST_W = VC - (NVT - 1) * NT  # 106
GROUPS = [(0, 4), (4, 4), (8, 4), (12, 1)]  # (start vt, count)
RG = [list(range(NCORES))]
STREAM_BUFS = int(os.environ.get("KERNEL_STREAM_BUFS", "20"))
G3 = 3 * HC

# column offsets inside the packed [128, 8320] f32 front blob
PF_XT, PF_HT, PF_CW, PF_WIH, PF_WHH, PF_END = 0, 64, 128, 2176, 5248, 8320
# column offsets inside the packed [8, 1096] f32 bias blob
PB_AB, PB_CB, PB_BIH, PB_BHH, PB_HSL, PB_EYE, PB_END = 0, 64, 192, 576, 960, 1088, 1096

_CACHE = {}


def _build():
    nc = bacc.Bacc(None, target_bir_lowering=False, num_devices=NCORES)

    dp = nc.declare_dram_parameter
    d_pf = dp("pack_f32", [128, PF_END], F32, isOutput=False)
    d_pb = dp("bias_pack", [B, PB_END], F32, isOutput=False)
    d_aw = dp("attn_wT", [128, 16 * LC], BF16, isOutput=False)
    d_enc = dp("enc", [LC, B * H], BF16, isOutput=False)
    d_sm = dp("small_bf", [B, 16], BF16, isOutput=False)
    d_owT = dp("out_wT", [H, VC], BF16, isOutput=False)
    d_ob = dp("out_b", [1, VC], BF16, isOutput=False)

    d_logp = dp("logp_out", [B, VC], F32, isOutput=True)
    d_hnew = dp("hnew_out", [B, HC], F32, isOutput=True)

    with tile.TileContext(nc) as tc:
        with (
            tc.tile_pool(name="const", bufs=1) as cpool,
            tc.tile_pool(name="work", bufs=1) as wpool,
            tc.tile_pool(name="stream", bufs=STREAM_BUFS) as spool,
            tc.tile_pool(name="psum", bufs=5, space="PSUM") as ppool,
            tc.tile_pool(name="papp", bufs=1, space="PSUM") as apool,
            tc.tile_pool(name="psmall", bufs=2, space="PSUM") as pspool,
            tc.tile_pool(name="dram", bufs=1, space="DRAM") as dpool,
        ):
            sdma = nc.sync.dma_start  # big stream (HWDGE)
            fdma = nc.gpsimd.dma_start  # front/small transfers (SWDGE)

            # collective bounce buffers (DRAM, internal). Per-core block in
            # cc1 padded to 1152 = 9*128 rows so the AllGather output
            # factors as (c q p) for a single strided reload.
            cc1_in = dpool.tile([1152, B], F32)
            cc1_out = dpool.tile([NCORES * 1152, B], F32)
            cc2_in = dpool.tile([128, B], F32)
            cc2_out = dpool.tile([H, B], F32)
            cc3_in = dpool.tile([128, B], F32)
            cc3_out = dpool.tile([H, B], F32)
            cc4_in = dpool.tile([B, 1], F32)
            cc4_out = dpool.tile([NCORES * B, 1], F32)

            # zero cc1's pad rows (sim NaN checker; AG just copies bytes)
            zz_t = wpool.tile([128, B], F32)
            nc.vector.memset(zz_t[:, :], 0.0)
            fdma(cc1_in[1025:1152, :], zz_t[0:127, :])

            # ---- packed front loads ----
            pf_t = cpool.tile([128, PF_END], F32)
            fdma(pf_t[:, 0:PF_CW], d_pf[:, 0:PF_CW])  # x^T|h^T first (attn)
            pb_t = cpool.tile([B, PB_END], F32)
            fdma(pb_t[:, :], d_pb[:, :])
            aw_t = cpool.tile([128, 16 * LC], BF16)
            fdma(aw_t[:, :], d_aw[:, :])
            enc_t = cpool.tile([LC, B * H], BF16)
            fdma(enc_t[:, :], d_enc[:, :])
            sm_t = cpool.tile([B, 16], BF16)
            fdma(sm_t[:, :], d_sm[:, :])
            fdma(pf_t[:, PF_CW:PF_END], d_pf[:, PF_CW:PF_END])  # comb/GRU w
            outb_t = cpool.tile([1, VC], BF16)
            sdma(outb_t[:, :], d_ob[:, :])

            xT_t = pf_t[:, PF_XT:PF_XT + KC * B]
            hT_t = pf_t[:, PF_HT:PF_HT + KC * B]
            cw_t = pf_t[:, PF_CW:PF_CW + 16 * HC]
            wih_t = pf_t[:, PF_WIH:PF_WIH + KC * G3]
            whh_t = pf_t[:, PF_WHH:PF_WHH + KC * G3]
            ab_t = pb_t[:, PB_AB:PB_AB + LC]
            cb_t = pb_t[:, PB_CB:PB_CB + HC]
            bih_t = pb_t[:, PB_BIH:PB_BIH + G3]
            bhh_t = pb_t[:, PB_BHH:PB_BHH + G3]
            hsl_t = pb_t[:, PB_HSL:PB_HSL + HC]
            eyef_t = pb_t[:, PB_EYE:PB_EYE + B]
            eye_t = sm_t[:, 0:B]
            ones_t = sm_t[0:1, B:2 * B]

            # bf16 copies of x^T / h^T for the (bf16) attention matmuls
            xh_bf = cpool.tile([128, 2 * KC * B], BF16)
            nc.vector.tensor_copy(xh_bf[:, :], pf_t[:, 0:2 * KC * B])
            xT_bf = xh_bf[:, 0:KC * B]
            hT_bf = xh_bf[:, KC * B:2 * KC * B]

            # ================= attention =================
            with nc.named_scope("attn"):
                ps_attn = pspool.tile([B, LC], F32, tag="ps_small")
                for k in range(16):
                    lhsT = xT_bf if k < KC else hT_bf
                    kk = k % KC
                    nc.tensor.matmul(
                        ps_attn[:, :],
                        lhsT[:, kk * B:(kk + 1) * B],
                        aw_t[:, k * LC:(k + 1) * LC],
                        start=(k == 0),
                        stop=(k == 15),
                    )
                al_t = wpool.tile([B, LC], F32)
                nc.vector.tensor_add(al_t[:, :], ps_attn[:, :], ab_t[:, :])
                # e = exp(scores), den = sum_l e (no max: |scores| < ~40)
                e_t = wpool.tile([B, LC], BF16)
                den_t = wpool.tile([B, 1], F32)
                nc.scalar.activation(
                    e_t[:, :], al_t[:, :], AF.Exp, accum_out=den_t[:, :]
                )
                ps_eT = pspool.tile([LC, B], BF16, tag="ps_small")
                nc.tensor.transpose(ps_eT[:, :], e_t[:, :], eye_t[0:B, 0:B])
                eT_t = wpool.tile([LC, B], BF16)
                nc.vector.tensor_copy(eT_t[:, :], ps_eT[:, :])

                # partial applied^T: appT[h,b] = sum_l enc[b,l,h]*e[b,l]
                ps_appT = apool.tile([128, KC * B], F32)
                for b in range(B):
                    for ch in range(KC):
                        nc.tensor.matmul(
                            ps_appT[:, ch * B + b:ch * B + b + 1],
                            enc_t[:, b * H + ch * 128: b * H + (ch + 1) * 128],
                            eT_t[:, b:b + 1],
                            start=True,
                            stop=True,
                        )
                appT_t = wpool.tile([128, KC * B], F32)
                nc.vector.tensor_copy(appT_t[:, :], ps_appT[:, :])

                # hoisted AG-independent matmuls: run during the collective
                # init window. gh = h @ w_hh^T + b_hh and the x-half of the
                # combine matmul depend only on local inputs.
                ps_ghe = pspool.tile([B, G3], F32, tag="ps_small")
                for k in range(KC):
                    nc.tensor.matmul(
                        ps_ghe[:, :], hT_t[:, k * B:(k + 1) * B],
                        whh_t[:, k * G3:(k + 1) * G3],
                        start=(k == 0), stop=(k == KC - 1),
                    )
                gh_t = wpool.tile([B, G3], F32)
                nc.vector.tensor_add(gh_t[:, :], ps_ghe[:, :], bhh_t[:, :])
                ps_gxe = pspool.tile([B, HC], F32, tag="ps_small")
                for k in range(KC):
                    nc.tensor.matmul(
                        ps_gxe[:, :], xT_t[:, k * B:(k + 1) * B],
                        cw_t[:, k * HC:(k + 1) * HC],
                        start=(k == 0), stop=(k == KC - 1),
                    )
                gx_t = wpool.tile([B, HC], F32)
                nc.vector.tensor_copy(gx_t[:, :], ps_gxe[:, :])
                # AllGather partials, reduce locally (AG is much cheaper
                # than AllReduce on-chip)
                fdma(
                    cc1_in[0:H, :].rearrange("(k p) b -> p k b", p=128),
                    appT_t[:, :].rearrange("p (k b) -> p k b", k=KC),
                )
                fdma(cc1_in[H:H + 1, :], den_t[:, :])
                nc.gpsimd.collective_compute(
                    "AllGather", mybir.AluOpType.bypass, replica_groups=RG,
                    ins=[cc1_in.opt()], outs=[cc1_out.opt()],
                )
                agbuf = wpool.tile([128, NCORES * 72], F32)
                fdma(
                    agbuf[:, :].rearrange("p (cq b) -> p cq b", b=B),
                    cc1_out[:, :].rearrange("(cq p) b -> p cq b", p=128),
                )
                appT_g = wpool.tile([128, KC * B], F32)
                nc.vector.tensor_add(
                    appT_g[:, :], agbuf[:, 0:KC * B],
                    agbuf[:, 72:72 + KC * B],
                )
                for c in range(2, NCORES):
                    nc.vector.tensor_add(
                        appT_g[:, :], appT_g[:, :],
                        agbuf[:, c * 72:c * 72 + KC * B],
                    )
                den8_t = wpool.tile([B, NCORES], F32)
                fdma(
                    den8_t[:, :],
                    cc1_out.rearrange("(c r) b -> c r b", r=1152)[:, H, :]
                    .rearrange("c b -> b c"),
                )
                denl_t = wpool.tile([B, 1], F32)
                nc.vector.reduce_sum(
                    denl_t[:, :], den8_t[:, :], axis=mybir.AxisListType.X
                )
                recip_t = wpool.tile([B, 1], F32)
                nc.vector.reciprocal(recip_t[:, :], denl_t[:, :])


            # ================= combine + relu =================
            with nc.named_scope("comb"):
                ps_ga = pspool.tile([B, HC], F32, tag="ps_small")
                for k in range(KC):
                    nc.tensor.matmul(
                        ps_ga[:, :],
                        appT_g[:, k * B:(k + 1) * B],
                        cw_t[:, (KC + k) * HC:(KC + k + 1) * HC],
                        start=(k == 0),
                        stop=(k == KC - 1),
                    )
                gsc_t = wpool.tile([B, HC], F32)
                nc.vector.tensor_scalar_mul(
                    gsc_t[:, :], ps_ga[:, :], recip_t[:, :]
                )
                gpre_t = wpool.tile([B, HC], F32)
                nc.vector.tensor_add(gpre_t[:, :], gsc_t[:, :], gx_t[:, :])
                gpre2_t = wpool.tile([B, HC], F32)
                nc.vector.tensor_add(gpre2_t[:, :], gpre_t[:, :], cb_t[:, :])
                g_t = wpool.tile([B, HC], F32)
                nc.scalar.activation(g_t[:, :], gpre2_t[:, :], AF.Relu)

                ps_gT = pspool.tile([HC, B], F32, tag="ps_small")
                nc.tensor.transpose(ps_gT[:, :], g_t[:, :], eyef_t[0:B, 0:B])
                gT_sb = wpool.tile([HC, B], F32)
                nc.vector.tensor_copy(gT_sb[:, :], ps_gT[:, :])
                fdma(cc2_in[:, :], gT_sb[:, :])
                nc.gpsimd.collective_compute(
                    "AllGather", mybir.AluOpType.bypass, replica_groups=RG,
                    ins=[cc2_in.opt()], outs=[cc2_out.opt()],
                )
                gT_t = cpool.tile([128, KC * B], F32)
                fdma(
                    gT_t[:, :].rearrange("p (k b) -> p k b", k=KC),
                    cc2_out[:, :].rearrange("(k p) b -> p k b", p=128),
                )

            # ================= GRU =================
            with nc.named_scope("gru"):
                ps_gi = pspool.tile([B, G3], F32, tag="ps_small")
                for k in range(KC):
                    nc.tensor.matmul(
                        ps_gi[:, :], gT_t[:, k * B:(k + 1) * B],
                        wih_t[:, k * G3:(k + 1) * G3],
                        start=(k == 0), stop=(k == KC - 1),
                    )
                gi_t = wpool.tile([B, G3], F32)
                nc.vector.tensor_add(gi_t[:, :], ps_gi[:, :], bih_t[:, :])

                # r and z gates share one add + one sigmoid over [8, 256]
                rz_pre = wpool.tile([B, 2 * HC], F32)
                nc.vector.tensor_add(
                    rz_pre[:, :], gi_t[:, 0:2 * HC], gh_t[:, 0:2 * HC]
                )
                rz_t = wpool.tile([B, 2 * HC], F32)
                nc.scalar.activation(rz_t[:, :], rz_pre[:, :], AF.Sigmoid)
                rhn_t = wpool.tile([B, HC], F32)
                nc.vector.tensor_mul(
                    rhn_t[:, :], rz_t[:, 0:HC], gh_t[:, 2 * HC:G3]
                )
                npre_t = wpool.tile([B, HC], F32)
                nc.vector.tensor_add(
                    npre_t[:, :], gi_t[:, 2 * HC:G3], rhn_t[:, :]
                )
                n_t = wpool.tile([B, HC], F32)
                nc.scalar.activation(n_t[:, :], npre_t[:, :], AF.Tanh)
                # h_new = n + z * (h - n)
                d_t = wpool.tile([B, HC], F32)
                nc.vector.tensor_sub(d_t[:, :], hsl_t[:, :], n_t[:, :])
                zd_t = wpool.tile([B, HC], F32)
                nc.vector.tensor_mul(zd_t[:, :], rz_t[:, HC:2 * HC], d_t[:, :])
                hnew_t = wpool.tile([B, HC], F32)
                nc.vector.tensor_add(hnew_t[:, :], n_t[:, :], zd_t[:, :])
                fdma(d_hnew[:, :], hnew_t[:, :])

                # h_new^T shard -> AllGather (f32) -> bf16 chunks for stream
                ps_hT = pspool.tile([HC, B], F32, tag="ps_small")
                nc.tensor.transpose(ps_hT[:, :], hnew_t[:, :], eyef_t[0:B, 0:B])
                hT_sb = wpool.tile([HC, B], F32)
                nc.vector.tensor_copy(hT_sb[:, :], ps_hT[:, :])
                fdma(cc3_in[:, :], hT_sb[:, :])
                nc.gpsimd.collective_compute(
                    "AllGather", mybir.AluOpType.bypass, replica_groups=RG,
                    ins=[cc3_in.opt()], outs=[cc3_out.opt()],
                )
                hn32_t = cpool.tile([128, KC * B], F32)
                fdma(
                    hn32_t[:, :].rearrange("p (k b) -> p k b", k=KC),
                    cc3_out[:, :].rearrange("(k p) b -> p k b", p=128),
                )
                hnT_t = cpool.tile([128, KC * B], BF16)
                nc.vector.tensor_copy(hnT_t[:, :], hn32_t[:, :])

            # ================= vocab stream =================
            with nc.named_scope("warm"):
                ps_w = pspool.tile([B, NT], F32, tag="ps_small")
                for i in range(12):
                    nc.tensor.matmul(
                        ps_w[:, :], hT_sb[:, :], pf_t[:, PF_CW + i * NT:
                                                      PF_CW + (i + 1) * NT],
                        start=(i == 0), stop=(i == 11),
                    )
                wsink_t = wpool.tile([B, NT], F32)
                nc.vector.tensor_copy(wsink_t[:, :], ps_w[:, :])
            with nc.named_scope("stream"):
                logits_t = cpool.tile([B, VC], F32)
                stats_t = cpool.tile([B, NVT], F32)
                for (s, cnt) in GROUPS:
                    gw = min(VC, (s + cnt) * NT) - s * NT
                    ps = [
                        ppool.tile([B, NT], F32, tag="ps_l", bufs=5,
                                   name=f"ps{s}_{j}")
                        for j in range(cnt)
                    ]
                    for k in range(KC):
                        rhs_t = spool.tile([128, 4 * NT], BF16, tag="rhs")
                        sdma(
                            rhs_t[:, 0:gw],
                            d_owT[k * 128:(k + 1) * 128, s * NT:s * NT + gw],
                        )
                        for j in range(cnt):
                            w = min(NT, gw - j * NT)
                            nc.tensor.matmul(
                                ps[j][:, 0:w],
                                hnT_t[:, k * B:(k + 1) * B],
                                rhs_t[:, j * NT:j * NT + w],
                                start=(k == 0),
                                stop=False,
                            )
                    for j in range(cnt):
                        vt = s + j
                        w = NT if vt < NVT - 1 else LAST_W
                        # + out_b via a K=1 ones row; closes the psum group
                        nc.tensor.matmul(
                            ps[j][:, 0:w], ones_t[:, :],
                            outb_t[:, vt * NT:vt * NT + w],
                            start=False, stop=True,
                        )
                        nc.scalar.activation(
                            logits_t[:, vt * NT:vt * NT + w], ps[j][:, 0:w],
                            AF.Copy,
                        )
                        esc_t = wpool.tile([B, NT], F32, tag="esc", bufs=3)
                        nc.scalar.activation(
                            esc_t[:, 0:w], ps[j][:, 0:w], AF.Exp,
                            accum_out=stats_t[:, vt:vt + 1],
                        )

            # ================= softmax tail =================
            with nc.named_scope("tail"):
                denv_t = wpool.tile([B, 1], F32)
                nc.vector.reduce_sum(
                    denv_t[:, :], stats_t[:, :], axis=mybir.AxisListType.X
                )
                fdma(cc4_in[:, :], denv_t[:, :])
                nc.gpsimd.collective_compute(
                    "AllGather", mybir.AluOpType.bypass, replica_groups=RG,
                    ins=[cc4_in.opt()], outs=[cc4_out.opt()],
                )
                dall_t = wpool.tile([B, NCORES], F32)
                fdma(
                    dall_t[:, :],
                    cc4_out[:, :].rearrange("(c b) one -> b c one", b=B),
                )
                deng_t = wpool.tile([B, 1], F32)
                nc.vector.reduce_sum(
                    deng_t[:, :], dall_t[:, :], axis=mybir.AxisListType.X
                )
                logz_t = wpool.tile([B, 1], F32)
                nc.scalar.activation(logz_t[:, :], deng_t[:, :], AF.Ln)
                nc.vector.tensor_scalar_sub(
                    logits_t[:, :], logits_t[:, :], logz_t[:, :]
                )
                sdma(d_logp[:, :], logits_t[:, :])

    nc.compile()
    return nc


def _prep_inputs(input, hidden, encoder_outputs, emb, attn_w, attn_b,
                 comb_w, comb_b, w_ih, w_hh, b_ih, b_hh, out_w, out_b):
    import ml_dtypes

    f32 = np.float32
    bf16 = ml_dtypes.bfloat16
    c_ = np.ascontiguousarray

    def chunked(a, width):
        # [K*128, width] -> [128, K*width], chunk k at cols [k*w,(k+1)*w)
        k = a.shape[0] // 128
        return a.reshape(k, 128, -1).transpose(1, 0, 2).reshape(128, -1)

    ids = np.asarray(input).reshape(-1).astype(np.int64)
    x = np.asarray(emb)[ids].astype(f32)  # [B, H] embedding row gather
    h = np.asarray(hidden, dtype=f32)[0]  # [B, H]
    enc = np.asarray(encoder_outputs, dtype=f32)
    awT = np.asarray(attn_w, dtype=f32).T  # [2H, L]
    cwT = np.asarray(comb_w, dtype=f32).T  # [2H, H]
    wihT = np.asarray(w_ih, dtype=f32).T  # [H, 3H]
    whhT = np.asarray(w_hh, dtype=f32).T
    ab = np.asarray(attn_b, dtype=f32)
    cb = np.asarray(comb_b, dtype=f32)
    bih = np.asarray(b_ih, dtype=f32)
    bhh = np.asarray(b_hh, dtype=f32)
    ow = np.asarray(out_w, dtype=f32)
    ob = np.asarray(out_b, dtype=f32)

    xTc = chunked(c_(x.T), B)  # [128, 64]
    hTc = chunked(c_(h.T), B)

    small = np.zeros((B, 16), dtype=bf16)
    small[:, 0:B] = np.eye(B, dtype=bf16)
    small[0, B:16] = np.ones(B, dtype=bf16)

    def rep(v):
        return np.broadcast_to(v[None, :], (B, v.shape[0])).astype(f32)

    in_maps = []
    for c in range(NCORES):
        lsl = slice(c * LC, (c + 1) * LC)
        hsl = slice(c * HC, (c + 1) * HC)
        vsl = slice(c * VC, (c + 1) * VC)
        gcols = np.concatenate(
            [np.arange(g * H + c * HC, g * H + (c + 1) * HC) for g in range(3)]
        )
        pf = np.empty((128, PF_END), dtype=f32)
        pf[:, PF_XT:PF_HT] = xTc
        pf[:, PF_HT:PF_CW] = hTc
        pf[:, PF_CW:PF_WIH] = chunked(c_(cwT[:, hsl]), HC)
        pf[:, PF_WIH:PF_WHH] = chunked(c_(wihT[:, gcols]), G3)
        pf[:, PF_WHH:PF_END] = chunked(c_(whhT[:, gcols]), G3)

        pb = np.empty((B, PB_END), dtype=f32)
        pb[:, PB_AB:PB_CB] = rep(ab[lsl])
        pb[:, PB_CB:PB_BIH] = rep(cb[hsl])
        pb[:, PB_BIH:PB_BHH] = rep(bih[gcols])
        pb[:, PB_BHH:PB_HSL] = rep(bhh[gcols])
        pb[:, PB_HSL:PB_EYE] = h[:, hsl]
        pb[:, PB_EYE:PB_END] = np.eye(B, dtype=f32)

        in_maps.append({
            "pack_f32": pf,
            "bias_pack": pb,
            "attn_wT": c_(chunked(c_(awT[:, lsl]), LC).astype(bf16)),
            "enc": c_(enc[:, lsl, :].transpose(1, 0, 2)
                      .reshape(LC, B * H).astype(bf16)),
            "small_bf": small,
            "out_wT": c_(ow[vsl, :].T.astype(bf16)),
            "out_b": c_(ob[vsl][None, :].astype(bf16)),
        })
    return in_maps


def _enable_axon_ntff_hook():
    """The agent image's antenv lacks axon_hooks; shim it and install the
    ctypes NTFF hook so run_bass_kernel_spmd(trace=True) works under axon."""
    import types

    try:
        import antenv.axon_hooks  # noqa: F401
        return
    except ImportError:
        pass
    import antenv

    mod = types.ModuleType("antenv.axon_hooks")
    _h = [None]
    mod.get_axon_ntff_profile_hook = lambda: _h[0]
    mod.set_axon_ntff_profile_hook = lambda hook: _h.__setitem__(0, hook)
    sys.modules["antenv.axon_hooks"] = mod
    antenv.axon_hooks = mod
    try:
        from trn_agent_boot.trn_boot import _ntff_profile_via_ctypes

        hook = _ntff_profile_via_ctypes("/opt/axon/libaxon_pjrt.so")
        if hook is not None:
            mod.set_axon_ntff_profile_hook(hook)
    except Exception as e:  # profiling-only convenience; never fatal
        print("ntff hook install failed:", e)


def kernel(**inputs):
    if "nc" not in _CACHE:
        _CACHE["nc"] = _build()
    nc = _CACHE["nc"]
    in_maps = _prep_inputs(**inputs)

    trace = bool(os.environ.get("KERNEL_PROFILE"))
    if trace:
        _enable_axon_ntff_hook()
        import concourse.bass_utils as _bu

        _bu.upload_artifacts = lambda tmpdir: "local://" + tmpdir
    res = run_bass_kernel_spmd(
        nc,
        in_maps,
        core_ids=list(range(NCORES)),
        trace=trace,
        trace_cores=list(range(NCORES)) if trace else None,
    )
    _CACHE["last_result"] = res

    logp = np.concatenate(
        [res.results[c]["logp_out"] for c in range(NCORES)], axis=1
    )
    hnew = np.concatenate(
        [res.results[c]["hnew_out"] for c in range(NCORES)], axis=1
    )
    return logp, hnew[None, :, :]
